# revision 1
# baseline (speedup 1.0000x reference)
"""Trainium2 Bass kernel for single-head causal attention.

Problem: x[B=4,T=2048,C=1024] -> q,k,v = x@Wq/Wk/Wv [T,64] -> causal softmax(q k^T/sqrt(C)) @ v.

Sharding: 8 cores = 4 batches x 2 query-halves (sequence-parallel queries,
replicated weights). Each core computes K/V projections for the full
sequence and attention for its 1024 queries.

SPMD-uniform trick: the time axis of each core's x^T copy is permuted so the
core's OWN query half comes first (columns 0-1023), the other half after.
Then the causal block structure is identical on every core:
  - k-tiles j=0..7  (own half): lower-triangular blocks, diagonal gets a
    constant 128x128 triangular mask; blocks above the diagonal are skipped.
  - k-tiles j=8..15 (other half): full rectangle whose validity differs only
    by DATA: an exp-bias "gate" per core (0.0 => keep, -60 => exp ~ 0).
Softmax normalization is fused into the AV matmul by appending a ones column
to V (output row 64 = sum of exp); division happens host-side on gather.

Device layout: S^T = k_tile^T-stationary x q^T-moving so the softmax free
dim is q and P^T feeds AV directly with V-natural stationary (V transposed
on-device via the DMA xbar, bf16).
"""

import numpy as np
import ml_dtypes

B, T, C, H = 4, 2048, 1024, 64
TQ = 1024          # queries per core
NT = 2048          # kv length per core
NCH = C // 128     # 8 contraction chunks
NKT = NT // 128    # 16 k-tiles
SCALE = 1.0 / 32.0  # 1/sqrt(C)
VSTRIDE = 80       # bf16 cols per v' tile slot (64 v + 1 ones + pad, 32B-aligned)

_prog_cache = {}


def _build_program():
    import concourse.mybir as mybir
    from concourse import bacc
    from concourse.tile import TileContext

    fp32 = mybir.dt.float32
    bf16 = mybir.dt.bfloat16
    Exp = mybir.ActivationFunctionType.Exp

    nc = bacc.Bacc("TRN2", target_bir_lowering=False, debug=False)

    xt_d = nc.dram_tensor("xt", [C, NT], bf16, kind="ExternalInput")
    wqk_d = nc.dram_tensor("wqk", [C, 128], bf16, kind="ExternalInput")
    wv_d = nc.dram_tensor("wv", [C, H], bf16, kind="ExternalInput")
    gate_d = nc.dram_tensor("gate", [128, 1], fp32, kind="ExternalInput")
    tri_d = nc.dram_tensor("tri", [128, 128], bf16, kind="ExternalInput")
    idn_d = nc.dram_tensor("idn", [64, 64], bf16, kind="ExternalInput")
    out_d = nc.dram_tensor("outT", [H + 1, TQ], fp32, kind="ExternalOutput")

    with TileContext(nc) as tc:
        with (
            tc.tile_pool(name="xtp", bufs=1) as xt_pool,
            tc.tile_pool(name="cst", bufs=1) as cst,
            tc.tile_pool(name="prj", bufs=1) as prj,
            tc.tile_pool(name="ptp", bufs=8) as ptp,
            tc.tile_pool(name="psA", bufs=1, space="PSUM") as psA,
            tc.tile_pool(name="psB", bufs=1, space="PSUM") as psB,
            tc.tile_pool(name="psS", bufs=2, space="PSUM") as psS,
            tc.tile_pool(name="psO", bufs=2, space="PSUM") as psO,
        ):
            # constants / weights
            wqk_sb = cst.tile([128, NCH, 128], bf16, tag="wqk")
            nc.sync.dma_start(out=wqk_sb[:], in_=wqk_d.rearrange("(o p) m -> p o m", p=128))
            wv_sb = cst.tile([128, NCH, H], bf16, tag="wv")
            nc.sync.dma_start(out=wv_sb[:], in_=wv_d.rearrange("(o p) m -> p o m", p=128))
            gate_sb = cst.tile([128, 1], fp32, tag="gate")
            nc.sync.dma_start(out=gate_sb[:], in_=gate_d[:])
            tri_sb = cst.tile([128, 128], bf16, tag="tri")
            nc.sync.dma_start(out=tri_sb[:], in_=tri_d[:])
            idn_sb = cst.tile([64, 64], bf16, tag="idn")
            nc.sync.dma_start(out=idn_sb[:], in_=idn_d[:])

            # x^T chunks (C on partitions)
            xt_sb = []
            for c in range(NCH):
                t = xt_pool.tile([128, NT], bf16, tag=f"xt{c}")
                nc.sync.dma_start(out=t[:], in_=xt_d[c * 128:(c + 1) * 128, :])
                xt_sb.append(t)

            # persistent projection outputs
            qT_sb = prj.tile([64, TQ], fp32, tag="qT")
            kT_sb = prj.tile([64, NT], fp32, tag="kT")
            vT_sb = prj.tile([64, NT], bf16, tag="vT")
            vp_sb = prj.tile([128, NKT * VSTRIDE], bf16, tag="vp")
            o_sb = prj.tile([H + 1, TQ], fp32, tag="osb")

            # ones column (col 64 of each VSTRIDE block) for the l-row trick
            nc.vector.memset(
                vp_sb.rearrange("p (t c) -> p t c", c=VSTRIDE)[:, :, 64:65], 1.0
            )

            # PE warmup during the initial x^T DMA (copied to a dummy spot
            # so the verifier sees a reader)
            wq_flat = wqk_sb.rearrange("p o m -> p (o m)")
            scratch = psS.tile([128, 512], fp32, tag="s")
            for _ in range(8):
                nc.tensor.matmul(
                    scratch[:], wqk_sb[:, 0, :], wq_flat[:, 0:512],
                    start=True, stop=True,
                )
            nc.vector.tensor_copy(out=vp_sb[:, 0:64], in_=scratch[:, 0:64])

            # AV accumulators (allocated later, after the v transposes
            # borrow the psO slots)
            o_ps = []

            def emit_quarter(tq):
                """Project 512 time columns: q^T/k^T (packed) and v^T."""
                sl = slice(tq * 512, (tq + 1) * 512)
                qk_ps = psA.tile([128, 512], fp32, tag="qk")
                v_ps = psB.tile([64, 512], fp32, tag="pv")
                for c in range(NCH):
                    nc.tensor.matmul(
                        qk_ps[:], wqk_sb[:, c, :], xt_sb[c][:, sl],
                        start=(c == 0), stop=(c == NCH - 1),
                    )
                for c in range(NCH):
                    nc.tensor.matmul(
                        v_ps[:], wv_sb[:, c, :], xt_sb[c][:, sl],
                        start=(c == 0), stop=(c == NCH - 1),
                    )
                if tq < TQ // 512:
                    nc.vector.tensor_copy(out=qT_sb[:, sl], in_=qk_ps[0:64, :])
                nc.vector.tensor_copy(out=kT_sb[:, sl], in_=qk_ps[64:128, :])
                nc.vector.tensor_copy(out=vT_sb[:, sl], in_=v_ps[:])

            pt_tiles = {}

            def emit_S(j):
                """One k-tile: S^T matmul, exp (with gate bias), diagonal mask."""
                a0 = 128 * j if j < 8 else 0
                s_ps = psS.tile([128, 1024], fp32, tag="s")
                for b in (0, 1):
                    lo, hi = max(a0, 512 * b), 512 * (b + 1)
                    if lo < hi:
                        nc.tensor.matmul(
                            s_ps[:, lo:hi],
                            kT_sb[:, 128 * j: 128 * (j + 1)],
                            qT_sb[:, lo:hi],
                            start=True, stop=True,
                        )
                pt = ptp.tile([128, 1024], bf16, tag="pt")
                bias = gate_sb[:, 0:1] if j >= 8 else 0.0
                nc.scalar.activation(
                    pt[:, a0:1024], s_ps[:, a0:1024], Exp, bias=bias, scale=SCALE
                )
                if j < 8:
                    nc.vector.tensor_mul(
                        pt[:, 128 * j: 128 * (j + 1)],
                        pt[:, 128 * j: 128 * (j + 1)],
                        tri_sb[:],
                    )
                pt_tiles[j] = pt

            def emit_AV(j):
                a0 = 128 * j if j < 8 else 0
                pt = pt_tiles.pop(j)
                for b in (0, 1):
                    lo, hi = max(a0, 512 * b), 512 * (b + 1)
                    if lo < hi:
                        nc.tensor.matmul(
                            o_ps[b][:, lo - 512 * b: hi - 512 * b],
                            vp_sb[:, VSTRIDE * j: VSTRIDE * j + 65],
                            pt[:, lo:hi],
                            start=(j == 0), stop=(j == NKT - 1),
                            skip_group_check=True,
                        )

            emit_quarter(0)
            emit_quarter(1)
            emit_S(0)
            emit_S(1)
            emit_quarter(2)
            emit_S(2)
            emit_S(3)
            emit_quarter(3)
            emit_S(4)
            emit_S(5)

            # v^T -> v-natural via DMA xbar transpose (bf16)
            for t in range(NKT):
                nc.sync.dma_start(
                    out=vp_sb[:, VSTRIDE * t: VSTRIDE * t + 64],
                    in_=vT_sb[:, 128 * t: 128 * (t + 1)],
                    transpose=True,
                )

            o_ps0 = psO.tile([H + 1, 512], fp32, tag="o")
            o_ps1 = psO.tile([H + 1, 512], fp32, tag="o")
            o_ps.extend([o_ps0, o_ps1])

            for j in range(6):
                emit_AV(j)
            for j in range(6, NKT):
                emit_S(j)
                emit_AV(j)

            for b in (0, 1):
                nc.vector.tensor_copy(
                    out=o_sb[:, 512 * b: 512 * (b + 1)], in_=o_ps[b][:]
                )
            nc.sync.dma_start(out=out_d[:], in_=o_sb[:])

    nc.finalize()
    return nc


def _get_program():
    if "nc" not in _prog_cache:
        _prog_cache["nc"] = _build_program()
    return _prog_cache["nc"]


def make_in_maps(x, Wq, Wk, Wv):
    bf16 = ml_dtypes.bfloat16
    wqk = np.concatenate([Wq, Wk], axis=1).astype(bf16)  # [C, 128]
    wv = np.ascontiguousarray(Wv.astype(bf16))
    tri = np.triu(np.ones((128, 128), np.float32)).astype(bf16)  # tri[k,q]=1 iff q>=k
    idn = np.eye(64, dtype=np.float32).astype(bf16)
    in_maps = []
    for core in range(8):
        b, r = core // 2, core % 2
        qs = r * TQ
        other = (1 - r) * TQ
        xb = np.asarray(x[b])
        xt = np.concatenate([xb[qs:qs + TQ], xb[other:other + TQ]], axis=0).T
        gate = np.full((128, 1), 0.0 if r == 1 else -60.0, np.float32)
        in_maps.append({
            "xt": np.ascontiguousarray(xt).astype(bf16),
            "wqk": wqk,
            "wv": wv,
            "gate": gate,
            "tri": tri,
            "idn": idn,
        })
    return in_maps


def postprocess(results):
    out = np.empty((B, T, H), np.float32)
    for core in range(8):
        b, r = core // 2, core % 2
        qs = r * TQ
        oT = results[core]["outT"]  # [65, 1024]
        out[b, qs:qs + TQ] = (oT[:H] / oT[H:H + 1]).T
    return out


def kernel(x, mask, Wq, Wk, Wv, _trace=False, _tracedir=None):
    from concourse import bass_utils

    nc = _get_program()
    in_maps = make_in_maps(np.asarray(x, np.float32), np.asarray(Wq, np.float32),
                           np.asarray(Wk, np.float32), np.asarray(Wv, np.float32))
    res = bass_utils.run_bass_kernel_spmd(
        nc, in_maps, core_ids=list(range(8)),
        trace=_trace, tmpdir=_tracedir,
    )
    out = postprocess(res.results)
    if _trace:
        return out, res
    return out



# revision 2
# speedup vs baseline: 1.1822x; 1.1822x over previous
"""Trainium2 Bass kernel for single-head causal attention.

Problem: x[B=4,T=2048,C=1024] -> q,k,v = x@Wq/Wk/Wv [T,64] -> causal softmax(q k^T/sqrt(C)) @ v.

Sharding: 8 cores = 4 batches x 2 query-halves (sequence-parallel queries,
replicated weights). Each core computes K/V projections for the full
sequence and attention for its 1024 queries.

SPMD-uniform trick: the time axis of each core's x^T copy is permuted so the
core's OWN query half comes first (columns 0-1023), the other half after.
Then the causal block structure is identical on every core:
  - k-tiles j=0..7  (own half): lower-triangular blocks, diagonal gets a
    constant 128x128 triangular mask; blocks above the diagonal are skipped.
  - k-tiles j=8..15 (other half): full rectangle whose validity differs only
    by DATA: an exp-bias "gate" per core (0.0 => keep, -60 => exp ~ 0).
Softmax normalization is fused into the AV matmul by appending a ones column
to V (output row 64 = sum of exp); division happens host-side on gather.

v2 perf structure:
  - x is DMA'd in 4 column-slices of [C, 512] (rearranged to [128, 8, 512])
    so projections pipeline per-slice behind the DMA and the PE stays warm.
  - Projections are packed: [Wq|Wk] @ own half (q only needed for own
    queries), [Wk|Wv] @ other half, and Wv @ own half as col-tiled pairs
    (two 64-wide matmuls in col groups 0/1 run concurrently).
  - qT/kT are stored bf16 so the S^T matmuls run at bf16 rate (fp32 is 4
    cycles/row on the PE).
  - Output is evacuated and DMA'd per 512-column half to shorten the tail.
"""

import numpy as np
import ml_dtypes

B, T, C, H = 4, 2048, 1024, 64
TQ = 1024          # queries per core
NT = 2048          # kv length per core
NCH = C // 128     # 8 contraction chunks
NKT = NT // 128    # 16 k-tiles
NSL = 4            # 512-col time slices
SCALE = 1.0 / 32.0  # 1/sqrt(C)
VSTRIDE = 80       # bf16 cols per v' tile slot (64 v + 1 ones + pad, 32B-aligned)

_prog_cache = {}


def _build_program():
    import concourse.mybir as mybir
    from concourse import bacc
    from concourse.tile import TileContext

    fp32 = mybir.dt.float32
    bf16 = mybir.dt.bfloat16
    Exp = mybir.ActivationFunctionType.Exp

    nc = bacc.Bacc("TRN2", target_bir_lowering=False, debug=False)

    xt_d = nc.dram_tensor("xt", [C, NT], bf16, kind="ExternalInput")
    wqk_d = nc.dram_tensor("wqk", [C, 128], bf16, kind="ExternalInput")
    wkv_d = nc.dram_tensor("wkv", [C, 128], bf16, kind="ExternalInput")
    wv_d = nc.dram_tensor("wv", [C, H], bf16, kind="ExternalInput")
    gate_d = nc.dram_tensor("gate", [128, 1], fp32, kind="ExternalInput")
    tri_d = nc.dram_tensor("tri", [128, 128], bf16, kind="ExternalInput")
    out_d = nc.dram_tensor("outT", [H + 1, TQ], fp32, kind="ExternalOutput")

    with TileContext(nc) as tc:
        with (
            tc.tile_pool(name="xtp", bufs=1) as xt_pool,
            tc.tile_pool(name="cst", bufs=1) as cst,
            tc.tile_pool(name="prj", bufs=1) as prj,
            tc.tile_pool(name="ptp", bufs=8) as ptp,
            tc.tile_pool(name="psA", bufs=1, space="PSUM") as psA,
            tc.tile_pool(name="psB", bufs=1, space="PSUM") as psB,
            tc.tile_pool(name="psS", bufs=2, space="PSUM") as psS,
            tc.tile_pool(name="psO", bufs=2, space="PSUM") as psO,
        ):
            # warmup fodder first: memset is ready instantly, so the PE can
            # start chewing before any DMA lands (keeps the HAM clock warm)
            wrm_sb = cst.tile([128, 512], bf16, tag="wrm")
            nc.vector.memset(wrm_sb[:], 0.0)

            # x^T slices: [128 part, 8 chunks, 512 cols] each
            xt_sl = []
            for s in range(NSL):
                t = xt_pool.tile([128, NCH, 512], bf16, tag=f"xs{s}")
                eng = nc.sync if s % 2 == 0 else nc.scalar
                eng.dma_start(
                    out=t[:],
                    in_=xt_d[:, 512 * s: 512 * (s + 1)].rearrange(
                        "(o p) m -> p o m", p=128
                    ),
                )
                xt_sl.append(t)

            def xt(c, sl):
                """chunk c of 512-col slice sl -> [128, 512] view"""
                return xt_sl[sl][:, c, :]

            # constants / weights
            wqk_sb = cst.tile([128, NCH, 128], bf16, tag="wqk")
            nc.sync.dma_start(out=wqk_sb[:], in_=wqk_d.rearrange("(o p) m -> p o m", p=128))
            wkv_sb = cst.tile([128, NCH, 128], bf16, tag="wkv")
            nc.sync.dma_start(out=wkv_sb[:], in_=wkv_d.rearrange("(o p) m -> p o m", p=128))
            wv_sb = cst.tile([128, NCH, H], bf16, tag="wv")
            nc.sync.dma_start(out=wv_sb[:], in_=wv_d.rearrange("(o p) m -> p o m", p=128))
            gate_sb = cst.tile([128, 1], fp32, tag="gate")
            nc.sync.dma_start(out=gate_sb[:], in_=gate_d[:])
            tri_sb = cst.tile([128, 128], bf16, tag="tri")
            nc.sync.dma_start(out=tri_sb[:], in_=tri_d[:])

            # persistent projection outputs (bf16 so S^T runs at bf16 rate)
            qT_sb = prj.tile([64, TQ], bf16, tag="qT")
            kT_sb = prj.tile([64, NT], bf16, tag="kT")
            vT_sb = prj.tile([64, NT], bf16, tag="vT")
            vp_sb = prj.tile([128, NKT * VSTRIDE], bf16, tag="vp")
            o_sb = prj.tile([H + 1, TQ], fp32, tag="osb")

            # ones column (col 64 of each VSTRIDE block) for the l-row trick
            nc.vector.memset(
                vp_sb.rearrange("p (t c) -> p t c", c=VSTRIDE)[:, :, 64:65], 1.0
            )

            # PE warmup on the memset tile (no DMA dependency). Copied to a
            # dummy spot so the verifier sees a reader; vp[:, 0:64] is
            # overwritten by the j=0 transpose later.
            scratch = psS.tile([128, 512], fp32, tag="s")
            for i in range(10):
                nc.tensor.matmul(
                    scratch[:], wrm_sb[:, 0:128], wrm_sb[:],
                    start=(i == 0), stop=(i == 9),
                )
            nc.vector.tensor_copy(out=vp_sb[:, 0:64], in_=scratch[:, 0:64])

            o_ps = []

            def emit_qk(sl):
                """[Wq|Wk] @ own-half slice sl: q (rows 0-63) + k (rows 64-127)."""
                qk_ps = psA.tile([128, 512], fp32, tag="qk")
                for c in range(NCH):
                    nc.tensor.matmul(
                        qk_ps[:], wqk_sb[:, c, :], xt(c, sl),
                        start=(c == 0), stop=(c == NCH - 1),
                    )
                cs = slice(512 * sl, 512 * (sl + 1))
                nc.vector.tensor_copy(out=qT_sb[:, cs], in_=qk_ps[0:64, :])
                nc.vector.tensor_copy(out=kT_sb[:, cs], in_=qk_ps[64:128, :])

            def emit_kv(sl):
                """[Wk|Wv] @ other-half slice sl: k (rows 0-63) + v (rows 64-127)."""
                kv_ps = psA.tile([128, 512], fp32, tag="qk")
                for c in range(NCH):
                    nc.tensor.matmul(
                        kv_ps[:], wkv_sb[:, c, :], xt(c, sl),
                        start=(c == 0), stop=(c == NCH - 1),
                    )
                cs = slice(512 * sl, 512 * (sl + 1))
                nc.vector.tensor_copy(out=kT_sb[:, cs], in_=kv_ps[0:64, :])
                nc.vector.tensor_copy(out=vT_sb[:, cs], in_=kv_ps[64:128, :])

            def emit_v_own():
                """Wv @ own half as col-tiled concurrent pairs:
                col-group 0 accumulates slice 0, col-group 1 slice 1."""
                v_ps = psB.tile([128, 512], fp32, tag="pv")
                for c in range(NCH):
                    nc.tensor.matmul(
                        v_ps[0:64, :], wv_sb[:, c, :], xt(c, 0),
                        start=(c == 0), stop=(c == NCH - 1),
                        tile_position=(0, 0),
                    )
                    nc.tensor.matmul(
                        v_ps[64:128, :], wv_sb[:, c, :], xt(c, 1),
                        start=(c == 0), stop=(c == NCH - 1),
                        tile_position=(0, 64),
                    )
                nc.vector.tensor_copy(out=vT_sb[:, 0:512], in_=v_ps[0:64, :])
                nc.vector.tensor_copy(out=vT_sb[:, 512:1024], in_=v_ps[64:128, :])

            pt_tiles = {}

            def emit_S(j):
                """One k-tile: S^T matmul (bf16), exp (with gate bias), diag mask."""
                a0 = 128 * j if j < 8 else 0
                s_ps = psS.tile([128, 1024], fp32, tag="s")
                for b in (0, 1):
                    lo, hi = max(a0, 512 * b), 512 * (b + 1)
                    if lo < hi:
                        nc.tensor.matmul(
                            s_ps[:, lo:hi],
                            kT_sb[:, 128 * j: 128 * (j + 1)],
                            qT_sb[:, lo:hi],
                            start=True, stop=True,
                        )
                pt = ptp.tile([128, 1024], bf16, tag="pt")
                bias = gate_sb[:, 0:1] if j >= 8 else 0.0
                nc.scalar.activation(
                    pt[:, a0:1024], s_ps[:, a0:1024], Exp, bias=bias, scale=SCALE
                )
                if j < 8:
                    nc.vector.tensor_mul(
                        pt[:, 128 * j: 128 * (j + 1)],
                        pt[:, 128 * j: 128 * (j + 1)],
                        tri_sb[:],
                    )
                pt_tiles[j] = pt

            def emit_AV(j):
                a0 = 128 * j if j < 8 else 0
                pt = pt_tiles.pop(j)
                for b in (0, 1):
                    lo, hi = max(a0, 512 * b), 512 * (b + 1)
                    if lo < hi:
                        nc.tensor.matmul(
                            o_ps[b][:, lo - 512 * b: hi - 512 * b],
                            vp_sb[:, VSTRIDE * j: VSTRIDE * j + 65],
                            pt[:, lo:hi],
                            start=(j == 0), stop=(j == NKT - 1),
                            skip_group_check=True,
                        )

            def emit_vtr(t):
                """v^T tile t -> v-natural via DMA xbar transpose (bf16)."""
                nc.sync.dma_start(
                    out=vp_sb[:, VSTRIDE * t: VSTRIDE * t + 64],
                    in_=vT_sb[:, 128 * t: 128 * (t + 1)],
                    transpose=True,
                )

            # ---- schedule ----
            emit_qk(0)
            emit_qk(1)
            emit_v_own()
            for t in range(4):
                emit_vtr(t)
            emit_S(0)
            emit_S(1)
            emit_kv(2)
            for t in range(4, 8):
                emit_vtr(t)
            emit_S(2)
            emit_S(3)
            emit_kv(3)
            emit_S(4)
            emit_S(5)
            for t in range(8, NKT):
                emit_vtr(t)

            o_ps0 = psO.tile([H + 1, 512], fp32, tag="o")
            o_ps1 = psO.tile([H + 1, 512], fp32, tag="o")
            o_ps.extend([o_ps0, o_ps1])

            for j in range(6):
                emit_AV(j)
            for j in range(6, NKT):
                emit_S(j)
                emit_AV(j)

            for b in (0, 1):
                nc.vector.tensor_copy(
                    out=o_sb[:, 512 * b: 512 * (b + 1)], in_=o_ps[b][:]
                )
                nc.sync.dma_start(
                    out=out_d[:, 512 * b: 512 * (b + 1)],
                    in_=o_sb[:, 512 * b: 512 * (b + 1)],
                )

    nc.finalize()
    return nc


def _get_program():
    if "nc" not in _prog_cache:
        _prog_cache["nc"] = _build_program()
    return _prog_cache["nc"]


def make_in_maps(x, Wq, Wk, Wv):
    bf16 = ml_dtypes.bfloat16
    wqk = np.concatenate([Wq, Wk], axis=1).astype(bf16)  # [C, 128]
    wkv = np.concatenate([Wk, Wv], axis=1).astype(bf16)  # [C, 128]
    wv = np.ascontiguousarray(Wv.astype(bf16))
    tri = np.triu(np.ones((128, 128), np.float32)).astype(bf16)  # tri[k,q]=1 iff q>=k
    in_maps = []
    for core in range(8):
        b, r = core // 2, core % 2
        qs = r * TQ
        other = (1 - r) * TQ
        xb = np.asarray(x[b])
        xt = np.concatenate([xb[qs:qs + TQ], xb[other:other + TQ]], axis=0).T
        gate = np.full((128, 1), 0.0 if r == 1 else -60.0, np.float32)
        in_maps.append({
            "xt": np.ascontiguousarray(xt).astype(bf16),
            "wqk": wqk,
            "wkv": wkv,
            "wv": wv,
            "gate": gate,
            "tri": tri,
        })
    return in_maps


def postprocess(results):
    out = np.empty((B, T, H), np.float32)
    for core in range(8):
        b, r = core // 2, core % 2
        qs = r * TQ
        oT = results[core]["outT"]  # [65, 1024]
        out[b, qs:qs + TQ] = (oT[:H] / oT[H:H + 1]).T
    return out


def kernel(x, mask, Wq, Wk, Wv, _trace=False, _tracedir=None):
    from concourse import bass_utils

    nc = _get_program()
    in_maps = make_in_maps(np.asarray(x, np.float32), np.asarray(Wq, np.float32),
                           np.asarray(Wk, np.float32), np.asarray(Wv, np.float32))
    res = bass_utils.run_bass_kernel_spmd(
        nc, in_maps, core_ids=list(range(8)),
        trace=_trace, tmpdir=_tracedir,
    )
    out = postprocess(res.results)
    if _trace:
        return out, res
    return out
